# revision 4
# baseline (speedup 1.0000x reference)
"""Causal multi-head attention block (B=4, S=2048, NX=1024, H=16, D=64)
distributed over 8 TRN2 NeuronCores.

Sharding: core i handles batch b = i//2 and head-group hg = i%2 (8 of 16
heads).  Each core computes qkv for its heads, causal attention, and a
partial c_proj over its 512 feature rows; the per-batch pair of cores
reduces partials with an on-chip ReduceScatter (or on the host).

All matmuls run in bf16 (f32 PSUM accumulate).  Attention is computed in
the transposed orientation s^T[k, q] = k @ q^T so softmax sums come from
an ones-augmented v matmul and a^T feeds c_proj's lhsT directly.
"""
import sys

sys.path.insert(0, "/opt/trn_rl_repo")

import functools

import ml_dtypes
import numpy as np

from concourse import bacc, mybir, tile
from concourse.bass_utils import run_bass_kernel_spmd

B, S, NX = 4, 2048, 1024
H, D = 16, 64
N_CORES = 8
HL = H // 2          # heads per core (local)
FL = HL * D          # local head feature width (512)
BF16 = mybir.dt.bfloat16
F32 = mybir.dt.float32
BF = ml_dtypes.bfloat16

NK = S // 128        # 16 k-tiles of 128
NQC = S // 512       # 4 q-chunks of 512
KK = NX // 128       # 8 contraction blocks

# "rs": on-chip ReduceScatter over core pairs; "host": partials summed on host
REDUCE_MODE = "rs"


def _build(reduce_mode: str):
    nc = bacc.Bacc("TRN2", target_bir_lowering=False, debug=False,
                   num_devices=N_CORES)

    xT_ext = nc.dram_tensor("xT", [NX, S], BF16, kind="ExternalInput")
    wqk_ext = nc.dram_tensor("w_qk", [NX, 2 * FL], BF16, kind="ExternalInput")
    wv_ext = nc.dram_tensor("w_v", [NX, FL], BF16, kind="ExternalInput")
    wp_ext = nc.dram_tensor("w_proj", [FL, NX], BF16, kind="ExternalInput")
    bqk_ext = nc.dram_tensor("b_qk", [2 * FL, 1], F32, kind="ExternalInput")
    bv_ext = nc.dram_tensor("bv_row", [1, FL], BF16, kind="ExternalInput")
    bp_ext = nc.dram_tensor("bp_row", [1, NX], BF16, kind="ExternalInput")
    if reduce_mode == "rs":
        out_ext = nc.dram_tensor("out", [S // 2, NX], F32, kind="ExternalOutput")
    else:
        out_ext = nc.dram_tensor("out", [S, NX], F32, kind="ExternalOutput")

    with tile.TileContext(nc) as tc:
        with tc.tile_pool(name="const", bufs=1) as cp, \
             tc.tile_pool(name="work", bufs=3) as wp, \
             tc.tile_pool(name="psS", bufs=2, space="PSUM") as psS, \
             tc.tile_pool(name="psU", bufs=2, space="PSUM") as psU, \
             tc.tile_pool(name="dram", bufs=1, space="DRAM") as dp:

            # ---- persistent SBUF tensors ----
            xT = cp.tile([128, KK, S], BF16, tag="xT")
            wqk = cp.tile([128, KK, 2 * FL], BF16, tag="wqk")
            wv = cp.tile([128, KK, FL], BF16, tag="wv")
            wproj = cp.tile([128, FL // 128, NX], BF16, tag="wproj")
            qkT = cp.tile([128, 2 * FL // 128, S], BF16, tag="qkT")
            v_aug = cp.tile([128, NK, HL, D + 1], BF16, tag="vaug")
            aT = cp.tile([128, FL // 128, S], BF16, tag="aT")
            bqk = cp.tile([128, 2 * FL // 128], F32, tag="bqk")
            bv_row = cp.tile([1, FL], BF16, tag="bv")
            bp_row = cp.tile([1, NX], BF16, tag="bp")
            ones_row = cp.tile([1, 128], BF16, tag="ones")
            tri = cp.tile([128, 128], BF16, tag="tri")

            # ---- input DMAs ----
            for kk in range(KK):
                nc.sync.dma_start(out=xT[:, kk, :],
                                  in_=xT_ext.ap()[kk * 128:(kk + 1) * 128, :])
                nc.sync.dma_start(out=wqk[:, kk, :],
                                  in_=wqk_ext.ap()[kk * 128:(kk + 1) * 128, :])
                nc.sync.dma_start(out=wv[:, kk, :],
                                  in_=wv_ext.ap()[kk * 128:(kk + 1) * 128, :])
            for kt in range(FL // 128):
                nc.sync.dma_start(out=wproj[:, kt, :],
                                  in_=wp_ext.ap()[kt * 128:(kt + 1) * 128, :])
            for fb in range(2 * FL // 128):
                nc.sync.dma_start(out=bqk[:, fb:fb + 1],
                                  in_=bqk_ext.ap()[fb * 128:(fb + 1) * 128, :])
            nc.sync.dma_start(out=bv_row[:], in_=bv_ext.ap())
            nc.sync.dma_start(out=bp_row[:], in_=bp_ext.ap())

            nc.vector.memset(ones_row[:], 1.0)
            # tri[p, f] = 1 if p <= f else 0 (keep-in on p > f, else fill 1)
            nc.vector.memset(tri[:], 0.0)
            nc.gpsimd.affine_select(
                out=tri[:], in_=tri[:],
                compare_op=mybir.AluOpType.is_gt,
                fill=1.0, base=0, pattern=[[-1, 128]], channel_multiplier=1,
            )
            # ones column of v_aug
            nc.vector.memset(v_aug[:, :, :, D:D + 1], 1.0)

            if reduce_mode == "rs":
                rs_in = dp.tile([S, NX], F32)
                rs_out = dp.tile([S // 2, NX], F32)

            # ---- stage 2: v (natural layout) ----
            def emit_v(st):
                ps = psS.tile([128, FL], F32, tag="ps")
                for kk in range(KK):
                    nc.tensor.matmul(ps[:], xT[:, kk, st * 128:(st + 1) * 128],
                                     wv[:, kk, :], start=(kk == 0), stop=False)
                nc.tensor.matmul(ps[:], ones_row[:], bv_row[:],
                                 start=False, stop=True)
                nc.vector.tensor_copy(
                    v_aug[:, st, :, 0:D],
                    ps[:].rearrange("p (h d) -> p h d", d=D))

            # ---- stage 1: q^T / k^T (feature-major) ----
            def emit_qk(fb):
                for n0, nw in ((0, 1536), (1536, 512)):
                    ps = psS.tile([128, nw], F32, tag="ps")
                    for c0 in range(0, nw, 512):
                        for kk in range(KK):
                            nc.tensor.matmul(
                                ps[:, c0:c0 + 512],
                                wqk[:, kk, fb * 128:(fb + 1) * 128],
                                xT[:, kk, n0 + c0:n0 + c0 + 512],
                                start=(kk == 0), stop=(kk == KK - 1))
                    nc.vector.tensor_scalar_add(qkT[:, fb, n0:n0 + nw], ps[:],
                                                bqk[:, fb:fb + 1])

            for st in range(NK):
                emit_v(st)
            for fb in range(2):
                emit_qk(fb * (FL // 128))      # fb0 (q heads 0-1), fb4 (k heads 0-1)

            # ---- stage 3: attention per local head ----
            def emit_head(lh):
                fbq = lh // 2
                fbk = FL // 128 + lh // 2
                po = (lh % 2) * 64
                for qc in range(NQC):
                    qb = qc * 512
                    n_full = 4 * qc
                    # groups of k-tiles: full tiles in 3s, then the 4 diagonal
                    # tiles packed into one group (N = 512,384,256,128)
                    groups = []
                    kt0 = 0
                    while kt0 < n_full:
                        g = min(3, n_full - kt0)
                        groups.append([(kt0 + j, j * 512, 512, 0)
                                       for j in range(g)])
                        kt0 += g
                    # pack at bank-safe offsets: no matmul output may cross a
                    # 512-f32 PSUM bank boundary
                    diag_offs = (0, 512, 1024, 1280)
                    diag = [(n_full + j, diag_offs[j], 512 - 128 * j, 128 * j)
                            for j in range(4)]
                    groups.append(diag)

                    pu = psU.tile([65, 512], F32, tag="pu")
                    last_kt = n_full + 3
                    for gi, g in enumerate(groups):
                        is_diag = gi == len(groups) - 1
                        gw = max(off + N for (_, off, N, _) in g)
                        ps = psS.tile([128, 1536], F32, tag="ps")
                        for (kt, off, N, qoff) in g:
                            nc.tensor.matmul(
                                ps[:, off:off + N],
                                qkT[po:po + 64, fbk, kt * 128:(kt + 1) * 128],
                                qkT[po:po + 64, fbq, qb + qoff:qb + 512],
                                start=True, stop=True)
                        p = wp.tile([128, 1536], BF16, tag="p")
                        nc.scalar.activation(p[:, 0:gw], ps[:, 0:gw],
                                             mybir.ActivationFunctionType.Exp,
                                             scale=0.125)
                        if is_diag:
                            for (kt, off, N, qoff) in g:
                                nc.vector.tensor_mul(p[:, off:off + 128],
                                                     p[:, off:off + 128], tri[:])
                        for (kt, off, N, qoff) in g:
                            nc.tensor.matmul(
                                pu[:, qoff:qoff + N],
                                v_aug[:, kt, lh, :],
                                p[:, off:off + N],
                                start=(kt == 0), stop=(kt == last_kt))
                    # normalize: aT[d, q] = u[d, q] / u[64, q]
                    recip = wp.tile([1, 512], F32, tag="recip")
                    nc.vector.reciprocal(recip[:], pu[64:65, :])
                    rbc = wp.tile([64, 512], F32, tag="rbc")
                    nc.gpsimd.partition_broadcast(rbc[:], recip[:])
                    nc.vector.tensor_mul(aT[po:po + 64, fbq, qb:qb + 512],
                                         pu[0:64, :], rbc[:])

            for pair in range(4):
                if pair > 0:
                    emit_qk(pair)
                    emit_qk(FL // 128 + pair)
                emit_head(2 * pair)
                emit_head(2 * pair + 1)

            # ---- stage 4: c_proj partials ----
            for st in range(NK):
                for n0 in range(0, NX, 512):
                    ps = psS.tile([128, 512], F32, tag="ps")
                    for kt in range(FL // 128):
                        nc.tensor.matmul(ps[:], aT[:, kt, st * 128:(st + 1) * 128],
                                         wproj[:, kt, n0:n0 + 512],
                                         start=(kt == 0), stop=False)
                    nc.tensor.matmul(ps[:], ones_row[:], bp_row[:, n0:n0 + 512],
                                     start=False, stop=True)
                    osb = wp.tile([128, 512], F32, tag="osb")
                    nc.vector.tensor_copy(osb[:], ps[:])
                    if reduce_mode == "rs":
                        nc.sync.dma_start(
                            out=rs_in[st * 128:(st + 1) * 128, n0:n0 + 512],
                            in_=osb[:])
                    else:
                        nc.sync.dma_start(
                            out=out_ext.ap()[st * 128:(st + 1) * 128, n0:n0 + 512],
                            in_=osb[:])

            if reduce_mode == "rs":
                nc.gpsimd.collective_compute(
                    "ReduceScatter",
                    mybir.AluOpType.add,
                    replica_groups=[[0, 1], [2, 3], [4, 5], [6, 7]],
                    ins=[rs_in.opt()],
                    outs=[rs_out.opt()],
                )
                nc.sync.dma_start(out=out_ext.ap(), in_=rs_out[:])

    nc.compile()
    return nc


@functools.lru_cache(maxsize=2)
def _built(reduce_mode: str):
    return _build(reduce_mode)


def _in_maps(x, c_attn_w, c_attn_b, c_proj_w, c_proj_b, reduce_mode):
    maps = []
    for core in range(N_CORES):
        b, hg = core // 2, core % 2
        f0 = hg * FL
        w_q = c_attn_w[:, f0:f0 + FL]
        w_k = c_attn_w[:, NX + f0:NX + f0 + FL]
        w_v = c_attn_w[:, 2 * NX + f0:2 * NX + f0 + FL]
        b_q = c_attn_b[f0:f0 + FL]
        b_k = c_attn_b[NX + f0:NX + f0 + FL]
        b_v = c_attn_b[2 * NX + f0:2 * NX + f0 + FL]
        maps.append({
            "xT": np.ascontiguousarray(x[b].T).astype(BF),
            "w_qk": np.concatenate([w_q, w_k], axis=1).astype(BF),
            "w_v": np.ascontiguousarray(w_v).astype(BF),
            "w_proj": np.ascontiguousarray(c_proj_w[f0:f0 + FL, :]).astype(BF),
            "b_qk": np.concatenate([b_q, b_k]).astype(np.float32).reshape(-1, 1),
            "bv_row": b_v.astype(BF).reshape(1, FL),
            "bp_row": (c_proj_b / 2.0).astype(BF).reshape(1, NX),
        })
    return maps


def _run(inputs, reduce_mode=REDUCE_MODE, trace=False):
    nc = _built(reduce_mode)
    maps = _in_maps(inputs["x"], inputs["c_attn_w"], inputs["c_attn_b"],
                    inputs["c_proj_w"], inputs["c_proj_b"], reduce_mode)
    res = run_bass_kernel_spmd(nc, maps, core_ids=list(range(N_CORES)),
                               trace=trace)
    out = np.empty((B, S, NX), dtype=np.float32)
    for b in range(B):
        if reduce_mode == "rs":
            out[b, :S // 2] = res.results[2 * b]["out"]
            out[b, S // 2:] = res.results[2 * b + 1]["out"]
        else:
            out[b] = res.results[2 * b]["out"] + res.results[2 * b + 1]["out"]
    return out, res


def kernel(**inputs):
    out, _ = _run({k: np.asarray(v) for k, v in inputs.items()})
    return out
